# revision 5
# baseline (speedup 1.0000x reference)
"""Trainium2 Bass kernel for nn_MultiHeadLatentAttention_82068235092052.

Reference computation (B=2, S=4096, E=4096, H=32, D=128):
    q = hs @ wq.T + bq   -> [B,S,H,D]     (wq/bq are fp8-roundtripped fp32)
    k = hs @ wk.T + bk
    v = hs @ wv.T + bv
    (latent = hs @ wl.T + bl is computed but UNUSED -> skipped entirely)
    scores  = einsum('bshd,bstd->bsht', q, k) / sqrt(D)   # attention over HEADS per position
    probs   = softmax(scores, -1)
    context = einsum('bsht,bstd->bshd', probs, v).reshape(B,S,E)

Strategy: data-parallel over the 8192 positions across 8 cores (1024 each,
processed in 2 halves of 512). Per core, one fused bf16 matmul
W[12288,4096] x X^T produces q/k/v in feature-major layout [d, head, pos]
(each 128-row feature tile == one head), which feeds per-position 32x32
head-attention done with tile_position-packed PE matmuls + PE transposes.
Softmax normalization is a per-partition tensor_scalar; 1/sqrt(D) is folded
into the exp activation's scale.

Weights are exactly representable in bf16 (fp8 e4m3 subset), so the only
quantization error is the activations' fp32->bf16 rounding.
"""

import os
import sys

import numpy as np

sys.path.insert(0, "/opt/trn_rl_repo")

import ml_dtypes

import concourse.bacc as bacc
import concourse.bass as bass
import concourse.tile as tile
from concourse import mybir
from concourse.masks import make_identity

# Problem constants (hardcoded; kernel.py must be self-contained).
B, S, E = 2, 4096, 4096
H, D = 32, 128
P_TOT = B * S            # 8192 positions
N_CORES = 8
P_CORE = P_TOT // N_CORES  # 1024 positions per core
HALF = P_CORE // 2         # 512 positions per half
FT = 3 * H                 # 96 feature tiles (q, k, v concatenated)
KT = E // 128              # 32 contraction tiles

BF16 = mybir.dt.bfloat16
F32 = mybir.dt.float32

_CACHED_NC = None


def build_nc():
    """Build the per-core Bass program (same program on all 8 cores)."""
    nc = bacc.Bacc(
        "TRN2",
        target_bir_lowering=False,
        debug=False,
        enable_asserts=True,
        num_devices=1,
    )

    xt = nc.dram_tensor("xt", [128, KT, P_CORE], BF16, kind="ExternalInput").ap()
    wt = nc.dram_tensor("wt", [FT, 128, KT * 128], BF16, kind="ExternalInput").ap()
    bias = nc.dram_tensor("bias", [128, FT], F32, kind="ExternalInput").ap()
    ctx_out = nc.dram_tensor("ctx", [128, H, P_CORE], BF16, kind="ExternalOutput").ap()

    from contextlib import ExitStack

    with tile.TileContext(nc) as tc, ExitStack() as stack:
        const = stack.enter_context(tc.tile_pool(name="const", bufs=1))
        xtp = stack.enter_context(tc.tile_pool(name="xtp", bufs=1))
        qkvp = stack.enter_context(tc.tile_pool(name="qkvp", bufs=1))
        wp = stack.enter_context(tc.tile_pool(name="wp", bufs=2))
        ctp = stack.enter_context(tc.tile_pool(name="ctp", bufs=1))
        ap_pool = stack.enter_context(tc.tile_pool(name="attn", bufs=3))
        psum = stack.enter_context(tc.tile_pool(name="psum", bufs=8, space="PSUM"))

        identity = const.tile([128, 128], BF16)
        make_identity(nc, identity)
        bias_sb = const.tile([128, FT], F32)
        nc.sync.dma_start(bias_sb, bias)

        inv_sqrt_d = 1.0 / float(np.sqrt(D))

        for hf in range(2):
            # ---- projections: qkv[d, ft, p] = sum_i W[ft*128+d, i] * X[p, i] (+ bias)
            xt_sb = xtp.tile([128, KT, HALF], BF16, tag="xt")
            nc.sync.dma_start(xt_sb, xt[:, :, hf * HALF:(hf + 1) * HALF])
            qk_sb = qkvp.tile([128, 2 * H, HALF], BF16, tag="qk")
            v_sb = qkvp.tile([128, HALF, H], BF16, tag="v")

            for ft in range(FT):
                w_sb = wp.tile([128, KT, 128], BF16, tag="w")
                nc.sync.dma_start(
                    w_sb, wt[ft].rearrange("p (a b) -> p a b", a=KT)
                )
                ps = psum.tile([128, HALF], F32, tag="ps")
                for kt in range(KT):
                    nc.tensor.matmul(
                        ps,
                        lhsT=w_sb[:, kt, :],
                        rhs=xt_sb[:, kt, :],
                        start=(kt == 0),
                        stop=(kt == KT - 1),
                    )
                # bias add (per-partition scalar) + cast to bf16, PSUM -> SBUF
                if ft < 2 * H:
                    dst = qk_sb[:, ft, :]
                else:
                    dst = v_sb[:, :, ft - 2 * H]
                nc.vector.tensor_scalar(
                    out=dst,
                    in0=ps,
                    scalar1=bias_sb[:, ft:ft + 1],
                    scalar2=None,
                    op0=mybir.AluOpType.add,
                )

            # ---- attention: 512 positions in 128 groups of 4
            ct_sb = ctp.tile([128, H, HALF], BF16, tag="ct")
            for g in range(128):
                g4 = g * 4
                # scores[pos*32+h, t] = sum_d q[d, h, pos] * k[d, t, pos]
                scores = psum.tile([128, H], F32, tag="ps")
                for j in range(4):
                    nc.tensor.matmul(
                        scores[32 * j:32 * j + 32, :],
                        lhsT=qk_sb[:, 0:H, g4 + j],
                        rhs=qk_sb[:, H:2 * H, g4 + j],
                        start=True,
                        stop=True,
                        tile_position=(0, 32 * j),
                    )
                # exp((scores) / sqrt(D)) and per-row sum, in one ACT op
                exp_sb = ap_pool.tile([128, H], BF16, tag="exp")
                zsum = ap_pool.tile([128, 1], F32, tag="z")
                nc.scalar.activation(
                    exp_sb,
                    scores,
                    mybir.ActivationFunctionType.Exp,
                    scale=inv_sqrt_d,
                    accum_out=zsum,
                )
                zinv = ap_pool.tile([128, 1], F32, tag="zi")
                nc.vector.reciprocal(zinv, zsum)
                # per-32x32-block transpose -> probsT block j = position j's [t, h]
                probsT = ap_pool.tile([128, H], BF16, tag="pt")
                nc.vector.transpose(probsT, exp_sb)
                # V for 4 positions -> [pos*32+t, d] via PE transpose
                vt_ps = psum.tile([128, 128], BF16, tag="ps")
                nc.tensor.transpose(
                    vt_ps,
                    v_sb[:, g4:g4 + 4, :].opt(),
                    identity,
                )
                vt_sb = ap_pool.tile([128, 128], BF16, tag="vt")
                nc.scalar.copy(vt_sb, vt_ps)
                # ctx[pos*32+h, d] = sum_t probsT[t, h] * vt[t, d] (unnormalized)
                ctx_ps = psum.tile([128, 128], F32, tag="ps")
                for j in range(4):
                    nc.tensor.matmul(
                        ctx_ps[32 * j:32 * j + 32, :],
                        lhsT=probsT[32 * j:32 * j + 32, :],
                        rhs=vt_sb[32 * j:32 * j + 32, :],
                        start=True,
                        stop=True,
                        tile_position=(32 * j, 32 * j),
                    )
                # normalize by 1/Z (per-partition scalar) + cast to bf16
                ctx_sb = ap_pool.tile([128, 128], BF16, tag="cx")
                nc.vector.tensor_scalar(
                    out=ctx_sb,
                    in0=ctx_ps,
                    scalar1=zinv,
                    scalar2=None,
                    op0=mybir.AluOpType.mult,
                )
                # transpose back to feature-major [d, pos*32+h]
                ct_ps = psum.tile([128, 128], BF16, tag="ps")
                nc.tensor.transpose(ct_ps, ctx_sb, identity)
                nc.vector.tensor_copy(
                    out=ct_sb[:, :, g4:g4 + 4].rearrange("d h p -> d p h"),
                    in_=ct_ps,
                )
            nc.sync.dma_start(ctx_out[:, :, hf * HALF:(hf + 1) * HALF], ct_sb)

    nc.compile()
    return nc


def get_nc():
    global _CACHED_NC
    if _CACHED_NC is None:
        _CACHED_NC = build_nc()
    return _CACHED_NC


def prep_inputs(hidden_states, wq, bq, wk, bk, wv, bv):
    """Host-side layout prep. Returns (in_maps, None)."""
    bf16 = ml_dtypes.bfloat16

    # X^T tiled: [ipart, kt, p] with p the global position index
    xt_all = (
        np.ascontiguousarray(hidden_states.reshape(P_TOT, E).T)
        .astype(bf16)
        .reshape(KT, 128, P_TOT)
        .transpose(1, 0, 2)
    )  # [128, KT, 8192] (view)

    # Fused weight W[12288, 4096] -> W^T tiled [ft, ipart, kt*128 + f]
    wcat = np.concatenate([wq, wk, wv], axis=0)  # [3E, E]
    wt = (
        np.ascontiguousarray(wcat.T)
        .astype(bf16)
        .reshape(KT, 128, FT, 128)
        .transpose(2, 1, 0, 3)
    )
    wt = np.ascontiguousarray(wt).reshape(FT, 128, KT * 128)

    bias_cols = np.ascontiguousarray(
        np.concatenate([bq, bk, bv]).astype(np.float32).reshape(FT, 128).T
    )  # [128, FT]

    in_maps = []
    for c in range(N_CORES):
        xt_c = np.ascontiguousarray(xt_all[:, :, c * P_CORE:(c + 1) * P_CORE])
        in_maps.append({"xt": xt_c, "wt": wt, "bias": bias_cols})
    return in_maps


def assemble_output(ctx_per_core):
    """ctx_per_core: list of [128, H, P_CORE] bf16 arrays -> [B, S, E] fp32."""
    full = np.concatenate(ctx_per_core, axis=2)  # [d=128, h=32, p=8192]
    out = full.transpose(2, 1, 0).astype(np.float32)  # [p, h, d]
    return np.ascontiguousarray(out.reshape(B, S, E))


def kernel(**inputs):
    from concourse.bass_utils import run_bass_kernel_spmd

    nc = get_nc()
    in_maps = prep_inputs(
        inputs["hidden_states"],
        inputs["wq"], inputs["bq"],
        inputs["wk"], inputs["bk"],
        inputs["wv"], inputs["bv"],
    )
    res = run_bass_kernel_spmd(nc, in_maps, core_ids=list(range(N_CORES)))
    ctxs = [np.asarray(r["ctx"]).reshape(128, H, P_CORE) for r in res.results]
    return assemble_output(ctxs)


# revision 7
# speedup vs baseline: 1.1419x; 1.1419x over previous
"""Trainium2 Bass kernel for nn_MultiHeadLatentAttention_82068235092052.

Reference computation (B=2, S=4096, E=4096, H=32, D=128):
    q = hs @ wq.T + bq   -> [B,S,H,D]     (wq/bq are fp8-roundtripped fp32)
    k = hs @ wk.T + bk
    v = hs @ wv.T + bv
    (latent = hs @ wl.T + bl is computed but UNUSED -> skipped entirely)
    scores  = einsum('bshd,bstd->bsht', q, k) / sqrt(D)   # attention over HEADS per position
    probs   = softmax(scores, -1)
    context = einsum('bsht,bstd->bshd', probs, v).reshape(B,S,E)

Strategy: data-parallel over the 8192 positions across 8 cores (1024 each,
processed in 2 halves of 512). Per core, one fused bf16 matmul
W[12288,4096] x X^T produces q/k/v in feature-major layout [d, head, pos]
(each 128-row feature tile == one head), which feeds per-position 32x32
head-attention done with tile_position-packed PE matmuls + PE transposes.
Softmax normalization is a per-partition tensor_scalar; 1/sqrt(D) is folded
into the exp activation's scale.

Weights are exactly representable in bf16 (fp8 e4m3 subset), so the only
quantization error is the activations' fp32->bf16 rounding.
"""

import os
import sys

import numpy as np

sys.path.insert(0, "/opt/trn_rl_repo")

import ml_dtypes

import concourse.bacc as bacc
import concourse.bass as bass
import concourse.tile as tile
from concourse import mybir
from concourse.masks import make_identity

# Problem constants (hardcoded; kernel.py must be self-contained).
B, S, E = 2, 4096, 4096
H, D = 32, 128
P_TOT = B * S            # 8192 positions
N_CORES = 8
P_CORE = P_TOT // N_CORES  # 1024 positions per core
HALF = P_CORE // 2         # 512 positions per half
FT = 3 * H                 # 96 feature tiles (q, k, v concatenated)
KT = E // 128              # 32 contraction tiles

BF16 = mybir.dt.bfloat16
F32 = mybir.dt.float32

_CACHED_NC = None


def build_nc():
    """Build the per-core Bass program (same program on all 8 cores)."""
    nc = bacc.Bacc(
        "TRN2",
        target_bir_lowering=False,
        debug=False,
        enable_asserts=True,
        num_devices=1,
    )

    xt = nc.dram_tensor("xt", [128, KT, P_CORE], BF16, kind="ExternalInput").ap()
    wt = nc.dram_tensor("wt", [FT, 128, KT * 128], BF16, kind="ExternalInput").ap()
    bias = nc.dram_tensor("bias", [128, FT], F32, kind="ExternalInput").ap()
    ctx_out = nc.dram_tensor("ctx", [128, P_CORE, H], BF16, kind="ExternalOutput").ap()

    from contextlib import ExitStack

    with tile.TileContext(nc) as tc, ExitStack() as stack:
        const = stack.enter_context(tc.tile_pool(name="const", bufs=1))
        xtp = stack.enter_context(tc.tile_pool(name="xtp", bufs=1))
        qkvp = stack.enter_context(tc.tile_pool(name="qkvp", bufs=1))
        wp = stack.enter_context(tc.tile_pool(name="wp", bufs=2))
        ctp = stack.enter_context(tc.tile_pool(name="ctp", bufs=1))
        ap_pool = stack.enter_context(tc.tile_pool(name="attn", bufs=3))
        psum = stack.enter_context(tc.tile_pool(name="psum", bufs=8, space="PSUM"))

        identity = const.tile([128, 128], BF16)
        make_identity(nc, identity)
        bias_sb = const.tile([128, FT], F32)
        nc.sync.dma_start(bias_sb, bias)

        inv_sqrt_d = 1.0 / float(np.sqrt(D))

        for hf in range(2):
            # ---- projections: qkv[d, ft, p] = sum_i W[ft*128+d, i] * X[p, i] (+ bias)
            xt_sb = xtp.tile([128, KT, HALF], BF16, tag="xt")
            nc.sync.dma_start(xt_sb, xt[:, :, hf * HALF:(hf + 1) * HALF])
            qk_sb = qkvp.tile([128, 2 * H, HALF], BF16, tag="qk")
            v_sb = qkvp.tile([128, HALF, H], BF16, tag="v")

            for ft in range(FT):
                w_sb = wp.tile([128, KT, 128], BF16, tag="w")
                nc.sync.dma_start(
                    w_sb, wt[ft].rearrange("p (a b) -> p a b", a=KT)
                )
                ps = psum.tile([128, HALF], F32, tag="ps")
                for kt in range(KT):
                    nc.tensor.matmul(
                        ps,
                        lhsT=w_sb[:, kt, :],
                        rhs=xt_sb[:, kt, :],
                        start=(kt == 0),
                        stop=(kt == KT - 1),
                    )
                # bias add (per-partition scalar) + cast to bf16, PSUM -> SBUF
                if ft < 2 * H:
                    dst = qk_sb[:, ft, :]
                else:
                    dst = v_sb[:, :, ft - 2 * H]
                nc.vector.tensor_scalar(
                    out=dst,
                    in0=ps,
                    scalar1=bias_sb[:, ft:ft + 1],
                    scalar2=None,
                    op0=mybir.AluOpType.add,
                )

            # ---- attention: 512 positions in 32 blocks of 16 (4 groups of 4)
            # scores are written BLOCK-DIAGONALLY per group: position j's [h,t]
            # lands in diag block j of a [128,128] region (off-diag = -1e30 so
            # exp() zeroes it). The resulting block-diagonal probsT turns PV
            # into ONE standard K=128 matmul per group.
            ct_sb = ctp.tile([128, HALF, H], BF16, tag="ct")
            for blk in range(HALF // 16):
                p0 = blk * 16  # first position of block (within half)
                scores = psum.tile([128, 4, 128], F32, tag="ps")
                nc.vector.memset(scores, -1e30)
                for g in range(4):
                    for j in range(4):
                        nc.tensor.matmul(
                            scores[32 * j:32 * j + 32, g, 32 * j:32 * j + 32],
                            lhsT=qk_sb[:, 0:H, p0 + 4 * g + j],
                            rhs=qk_sb[:, H:2 * H, p0 + 4 * g + j],
                            start=True,
                            stop=True,
                            tile_position=(0, 32 * j),
                        )
                # exp(scores/sqrt(D)) for all 16 positions in one ACT op
                exp_sb = ap_pool.tile([128, 4, 128], BF16, tag="exp")
                nc.scalar.activation(
                    exp_sb,
                    scores,
                    mybir.ActivationFunctionType.Exp,
                    scale=inv_sqrt_d,
                )
                # row sums + reciprocal + normalize (batched over the block)
                zsum = ap_pool.tile([128, 4], F32, tag="z")
                nc.vector.tensor_reduce(
                    zsum, exp_sb, axis=mybir.AxisListType.X, op=mybir.AluOpType.add
                )
                zinv = ap_pool.tile([128, 4], F32, tag="zi")
                nc.vector.reciprocal(zinv, zsum)
                probs = ap_pool.tile([128, 4, 128], BF16, tag="pb")
                nc.vector.tensor_tensor(
                    probs,
                    exp_sb,
                    zinv[:, :, None].to_broadcast((128, 4, 128)),
                    mybir.AluOpType.mult,
                )
                # per-32x32-block transpose keeps the block-diagonal structure
                probsT = ap_pool.tile([128, 4, 128], BF16, tag="pt")
                nc.vector.transpose(probsT, probs)
                # V -> [pos*32+t, d] per group via PE transpose (4 per block)
                vt_ps = psum.tile([128, 4, 128], BF16, tag="ps")
                for g in range(4):
                    nc.tensor.transpose(
                        vt_ps[:, g, :],
                        v_sb[:, p0 + 4 * g:p0 + 4 * g + 4, :].opt(),
                        identity,
                    )
                vt_sb = ap_pool.tile([128, 4, 128], BF16, tag="vt")
                nc.scalar.copy(vt_sb, vt_ps)
                # PV: ctd[d, (pos,h)] = sum_{pos',t} vt[32pos'+t, d] probsT[32pos'+t, 32pos+h]
                # (block-diagonal probsT masks cross-position terms to zero)
                ctd = psum.tile([128, 4, 128], F32, tag="ps")
                for g in range(4):
                    nc.tensor.matmul(
                        ctd[:, g, :],
                        lhsT=vt_sb[:, g, :],
                        rhs=probsT[:, g, :],
                        start=True,
                        stop=True,
                    )
                # one contiguous copy out of PSUM for the whole block
                nc.scalar.copy(ct_sb[:, p0:p0 + 16, :], ctd)
            nc.sync.dma_start(ctx_out[:, hf * HALF:(hf + 1) * HALF, :], ct_sb)

    nc.compile()
    return nc


def get_nc():
    global _CACHED_NC
    if _CACHED_NC is None:
        _CACHED_NC = build_nc()
    return _CACHED_NC


def prep_inputs(hidden_states, wq, bq, wk, bk, wv, bv):
    """Host-side layout prep. Returns (in_maps, None)."""
    bf16 = ml_dtypes.bfloat16

    # X^T tiled: [ipart, kt, p] with p the global position index
    xt_all = (
        np.ascontiguousarray(hidden_states.reshape(P_TOT, E).T)
        .astype(bf16)
        .reshape(KT, 128, P_TOT)
        .transpose(1, 0, 2)
    )  # [128, KT, 8192] (view)

    # Fused weight W[12288, 4096] -> W^T tiled [ft, ipart, kt*128 + f]
    wcat = np.concatenate([wq, wk, wv], axis=0)  # [3E, E]
    wt = (
        np.ascontiguousarray(wcat.T)
        .astype(bf16)
        .reshape(KT, 128, FT, 128)
        .transpose(2, 1, 0, 3)
    )
    wt = np.ascontiguousarray(wt).reshape(FT, 128, KT * 128)

    bias_cols = np.ascontiguousarray(
        np.concatenate([bq, bk, bv]).astype(np.float32).reshape(FT, 128).T
    )  # [128, FT]

    in_maps = []
    for c in range(N_CORES):
        xt_c = np.ascontiguousarray(xt_all[:, :, c * P_CORE:(c + 1) * P_CORE])
        in_maps.append({"xt": xt_c, "wt": wt, "bias": bias_cols})
    return in_maps


def assemble_output(ctx_per_core):
    """ctx_per_core: list of [128, H, P_CORE] bf16 arrays -> [B, S, E] fp32."""
    full = np.concatenate(ctx_per_core, axis=1)  # [d=128, p=8192, h=32]
    out = full.transpose(1, 2, 0).astype(np.float32)  # [p, h, d]
    return np.ascontiguousarray(out.reshape(B, S, E))


def kernel(**inputs):
    from concourse.bass_utils import run_bass_kernel_spmd

    nc = get_nc()
    in_maps = prep_inputs(
        inputs["hidden_states"],
        inputs["wq"], inputs["bq"],
        inputs["wk"], inputs["bk"],
        inputs["wv"], inputs["bv"],
    )
    res = run_bass_kernel_spmd(nc, in_maps, core_ids=list(range(N_CORES)))
    ctxs = [np.asarray(r["ctx"]).reshape(128, P_CORE, H) for r in res.results]
    return assemble_output(ctxs)


# revision 10
# speedup vs baseline: 1.2981x; 1.1368x over previous
"""Trainium2 Bass kernel for nn_MultiHeadLatentAttention_82068235092052.

Reference computation (B=2, S=4096, E=4096, H=32, D=128):
    q = hs @ wq.T + bq   -> [B,S,H,D]     (wq/bq are fp8-roundtripped fp32)
    k = hs @ wk.T + bk
    v = hs @ wv.T + bv
    (latent = hs @ wl.T + bl is computed but UNUSED -> skipped entirely)
    scores  = einsum('bshd,bstd->bsht', q, k) / sqrt(D)   # attention over HEADS per position
    probs   = softmax(scores, -1)
    context = einsum('bsht,bstd->bshd', probs, v).reshape(B,S,E)

Strategy: data-parallel over the 8192 positions across 8 cores (1024 each,
processed in 2 halves of 512). Per core, one fused bf16 matmul
W[12288,4096] x X^T produces q/k/v in feature-major layout [d, head, pos]
(each 128-row feature tile == one head), which feeds per-position 32x32
head-attention done with tile_position-packed PE matmuls + PE transposes.
Softmax normalization is a per-partition tensor_scalar; 1/sqrt(D) is folded
into the exp activation's scale.

Weights are exactly representable in bf16 (fp8 e4m3 subset), so the only
quantization error is the activations' fp32->bf16 rounding.
"""

import os
import sys

import numpy as np

sys.path.insert(0, "/opt/trn_rl_repo")

import ml_dtypes

import concourse.bacc as bacc
import concourse.bass as bass
import concourse.tile as tile
from concourse import mybir
from concourse.masks import make_identity

# Problem constants (hardcoded; kernel.py must be self-contained).
B, S, E = 2, 4096, 4096
H, D = 32, 128
P_TOT = B * S            # 8192 positions
N_CORES = 8
P_CORE = P_TOT // N_CORES  # 1024 positions per core
HALF = P_CORE // 2         # 512 positions per half
FT = 3 * H                 # 96 feature tiles (q, k, v concatenated)
KT = E // 128              # 32 contraction tiles

BF16 = mybir.dt.bfloat16
F32 = mybir.dt.float32

_CACHED_NC = None


def build_nc():
    """Build the per-core Bass program (same program on all 8 cores)."""
    nc = bacc.Bacc(
        "TRN2",
        target_bir_lowering=False,
        debug=False,
        enable_asserts=True,
        num_devices=1,
    )

    xt = nc.dram_tensor("xt", [128, KT, P_CORE], BF16, kind="ExternalInput").ap()
    wt = nc.dram_tensor("wt", [FT, 128, KT * 128], BF16, kind="ExternalInput").ap()
    bias = nc.dram_tensor("bias", [128, FT], F32, kind="ExternalInput").ap()
    ctx_out = nc.dram_tensor("ctx", [128, P_CORE, H], BF16, kind="ExternalOutput").ap()

    from contextlib import ExitStack

    with tile.TileContext(nc) as tc, ExitStack() as stack:
        const = stack.enter_context(tc.tile_pool(name="const", bufs=1))
        xtp = stack.enter_context(tc.tile_pool(name="xtp", bufs=1))
        qkvp = stack.enter_context(tc.tile_pool(name="qkvp", bufs=1))
        wp = stack.enter_context(tc.tile_pool(name="wp", bufs=2))
        ctp = stack.enter_context(tc.tile_pool(name="ctp", bufs=1))
        ap_pool = stack.enter_context(tc.tile_pool(name="attn", bufs=3))
        psum = stack.enter_context(tc.tile_pool(name="psum", bufs=2, space="PSUM"))
        sc_pool = stack.enter_context(tc.tile_pool(name="scps", bufs=1, space="PSUM"))
        vt_pool = stack.enter_context(tc.tile_pool(name="vtps", bufs=2, space="PSUM"))
        ct_pool = stack.enter_context(tc.tile_pool(name="ctps", bufs=2, space="PSUM"))

        identity = const.tile([128, 128], BF16)
        make_identity(nc, identity)
        bias_sb = const.tile([128, FT], F32)
        nc.sync.dma_start(bias_sb, bias)

        inv_sqrt_d = 1.0 / float(np.sqrt(D))

        # two persistent block-diagonal score banks: off-diagonal -1e30 is
        # written once here and survives (QK only overwrites the diagonals)
        score_tiles = []
        for i in range(2):
            sct = sc_pool.tile([128, 4, 128], F32, tag=f"sc{i}")
            nc.vector.memset(sct, -1e30)
            score_tiles.append(sct)
        blk_counter = [0]

        for hf in range(2):
            # ---- projections: qkv[d, ft, p] = sum_i W[ft*128+d, i] * X[p, i] (+ bias)
            xt_sb = xtp.tile([128, KT, HALF], BF16, tag="xt")
            nc.sync.dma_start(xt_sb, xt[:, :, hf * HALF:(hf + 1) * HALF])
            qk_sb = qkvp.tile([128, 2 * H, HALF], BF16, tag="qk")
            v_sb = qkvp.tile([128, HALF, H], BF16, tag="v")

            for ft in range(FT):
                w_sb = wp.tile([128, KT, 128], BF16, tag="w")
                nc.sync.dma_start(
                    w_sb, wt[ft].rearrange("p (a b) -> p a b", a=KT)
                )
                ps = psum.tile([128, HALF], F32, tag="ps")
                for kt in range(KT):
                    nc.tensor.matmul(
                        ps,
                        lhsT=w_sb[:, kt, :],
                        rhs=xt_sb[:, kt, :],
                        start=(kt == 0),
                        stop=(kt == KT - 1),
                    )
                # bias add (per-partition scalar) + cast to bf16, PSUM -> SBUF
                if ft < 2 * H:
                    dst = qk_sb[:, ft, :]
                else:
                    dst = v_sb[:, :, ft - 2 * H]
                nc.vector.tensor_scalar(
                    out=dst,
                    in0=ps,
                    scalar1=bias_sb[:, ft:ft + 1],
                    scalar2=None,
                    op0=mybir.AluOpType.add,
                )

            # ---- attention: 512 positions in 32 blocks of 16 (4 groups of 4)
            # scores are written BLOCK-DIAGONALLY per group: position j's [h,t]
            # lands in diag block j of a [128,128] region (off-diag = -1e30 so
            # exp() zeroes it). The resulting block-diagonal probsT turns PV
            # into ONE standard K=128 matmul per group.
            ct_sb = ctp.tile([128, HALF, H], BF16, tag="ct")
            for blk in range(HALF // 16):
                p0 = blk * 16  # first position of block (within half)
                scores = score_tiles[blk_counter[0] % 2]
                blk_counter[0] += 1
                for g in range(4):
                    for j in range(4):
                        nc.tensor.matmul(
                            scores[32 * j:32 * j + 32, g, 32 * j:32 * j + 32],
                            lhsT=qk_sb[:, 0:H, p0 + 4 * g + j],
                            rhs=qk_sb[:, H:2 * H, p0 + 4 * g + j],
                            start=True,
                            stop=True,
                            tile_position=(0, 32 * j),
                        )
                # exp(scores/sqrt(D)) for all 16 positions in one ACT op
                exp_sb = ap_pool.tile([128, 4, 128], BF16, tag="exp")
                nc.scalar.activation(
                    exp_sb,
                    scores,
                    mybir.ActivationFunctionType.Exp,
                    scale=inv_sqrt_d,
                )
                # row sums + reciprocal + normalize (batched over the block)
                zsum = ap_pool.tile([128, 4], F32, tag="z")
                nc.vector.tensor_reduce(
                    zsum, exp_sb, axis=mybir.AxisListType.X, op=mybir.AluOpType.add
                )
                zinv = ap_pool.tile([128, 4], F32, tag="zi")
                nc.vector.reciprocal(zinv, zsum)
                probs = ap_pool.tile([128, 4, 128], BF16, tag="pb")
                nc.vector.tensor_tensor(
                    probs,
                    exp_sb,
                    zinv[:, :, None].to_broadcast((128, 4, 128)),
                    mybir.AluOpType.mult,
                )
                # per-32x32-block transpose keeps the block-diagonal structure
                probsT = ap_pool.tile([128, 4, 128], BF16, tag="pt")
                nc.vector.transpose(probsT, probs)
                # V -> [pos*32+t, d] per group via PE transpose (4 per block)
                vt_ps = vt_pool.tile([128, 4, 128], BF16, tag="vt")
                for g in range(4):
                    nc.tensor.transpose(
                        vt_ps[:, g, :],
                        v_sb[:, p0 + 4 * g:p0 + 4 * g + 4, :].opt(),
                        identity,
                    )
                vt_sb = ap_pool.tile([128, 4, 128], BF16, tag="vt")
                nc.scalar.copy(vt_sb, vt_ps)
                # PV: ctd[d, (pos,h)] = sum_{pos',t} vt[32pos'+t, d] probsT[32pos'+t, 32pos+h]
                # (block-diagonal probsT masks cross-position terms to zero)
                ctd = ct_pool.tile([128, 4, 128], F32, tag="ctd")
                for g in range(4):
                    nc.tensor.matmul(
                        ctd[:, g, :],
                        lhsT=vt_sb[:, g, :],
                        rhs=probsT[:, g, :],
                        start=True,
                        stop=True,
                    )
                # one contiguous copy out of PSUM for the whole block
                nc.scalar.copy(ct_sb[:, p0:p0 + 16, :], ctd)
            nc.sync.dma_start(ctx_out[:, hf * HALF:(hf + 1) * HALF, :], ct_sb)

    nc.compile()
    return nc


def get_nc():
    global _CACHED_NC
    if _CACHED_NC is None:
        _CACHED_NC = build_nc()
    return _CACHED_NC


def prep_inputs(hidden_states, wq, bq, wk, bk, wv, bv):
    """Host-side layout prep. Returns (in_maps, None)."""
    bf16 = ml_dtypes.bfloat16

    # X^T tiled: [ipart, kt, p] with p the global position index
    xt_all = (
        np.ascontiguousarray(hidden_states.reshape(P_TOT, E).T)
        .astype(bf16)
        .reshape(KT, 128, P_TOT)
        .transpose(1, 0, 2)
    )  # [128, KT, 8192] (view)

    # Fused weight W[12288, 4096] -> W^T tiled [ft, ipart, kt*128 + f]
    wcat = np.concatenate([wq, wk, wv], axis=0)  # [3E, E]
    wt = (
        np.ascontiguousarray(wcat.T)
        .astype(bf16)
        .reshape(KT, 128, FT, 128)
        .transpose(2, 1, 0, 3)
    )
    wt = np.ascontiguousarray(wt).reshape(FT, 128, KT * 128)

    bias_cols = np.ascontiguousarray(
        np.concatenate([bq, bk, bv]).astype(np.float32).reshape(FT, 128).T
    )  # [128, FT]

    in_maps = []
    for c in range(N_CORES):
        xt_c = np.ascontiguousarray(xt_all[:, :, c * P_CORE:(c + 1) * P_CORE])
        in_maps.append({"xt": xt_c, "wt": wt, "bias": bias_cols})
    return in_maps


def assemble_output(ctx_per_core):
    """ctx_per_core: list of [128, H, P_CORE] bf16 arrays -> [B, S, E] fp32."""
    full = np.concatenate(ctx_per_core, axis=1)  # [d=128, p=8192, h=32]
    out = full.transpose(1, 2, 0).astype(np.float32)  # [p, h, d]
    return np.ascontiguousarray(out.reshape(B, S, E))


def kernel(**inputs):
    from concourse.bass_utils import run_bass_kernel_spmd

    nc = get_nc()
    in_maps = prep_inputs(
        inputs["hidden_states"],
        inputs["wq"], inputs["bq"],
        inputs["wk"], inputs["bk"],
        inputs["wv"], inputs["bv"],
    )
    res = run_bass_kernel_spmd(nc, in_maps, core_ids=list(range(N_CORES)))
    ctxs = [np.asarray(r["ctx"]).reshape(128, P_CORE, H) for r in res.results]
    return assemble_output(ctxs)
